# revision 47
# baseline (speedup 1.0000x reference)
"""Trainium2 Bass kernel for nn_Attention_77043123355775.

Sharded GQA causal attention with RoPE: 8 NeuronCores as 2-way data
parallel (batch) x 4-way tensor parallel (heads). Each core computes its
4 Q heads / 2 KV heads for one batch entry and a partial output
projection (x[b] @ W)^T; the host sums the 4 partials per batch.

All matmuls run in plain bf16 with fp32 PSUM accumulation (one matmul
per logical GEMM; measured end-to-end rel err ~4e-3). Scores are
computed transposed (k on partitions) so the kernel needs no on-chip
transposes. Weights are resident in SBUF (loaded once); x streams per
512-token chunk, double-buffered across chunks.
"""
import math
import os
import sys

for _p in ("/opt/trn_rl_repo",):
    if _p not in sys.path:
        sys.path.insert(0, _p)

import ml_dtypes
import numpy as np

import concourse.bass as bass
import concourse.mybir as mybir
import concourse.tile as tile

from concourse.tile import add_dep_helper

dt = mybir.dt
AF = mybir.ActivationFunctionType


def build_attention_nc(S=2048, D=2048, NQ=4, NKV=2, HD=128, TC=512):
    assert HD == 128
    C = D // 128          # contraction chunks over features
    TB = S // 128         # 128-token blocks
    NTC = S // TC         # token chunks
    DB = D // 128         # output feature blocks
    CO = NQ * HD // 128   # contraction chunks for wo (= NQ)
    REP = NQ // NKV
    scale = 1.0 / math.sqrt(HD)

    nc = bass.Bass()

    xt = nc.dram_tensor("xt", [D, S], dt.bfloat16, kind="ExternalInput")
    wqp = nc.dram_tensor("wqp", [D, NQ * HD], dt.bfloat16, kind="ExternalInput")
    wkp = nc.dram_tensor("wkp", [D, NKV * HD], dt.bfloat16, kind="ExternalInput")
    wvp = nc.dram_tensor("wvp", [D, NKV * HD], dt.bfloat16, kind="ExternalInput")
    woh = nc.dram_tensor("woh", [NQ * HD, D], dt.bfloat16, kind="ExternalInput")
    csT = nc.dram_tensor("csT", [HD, S], dt.float32, kind="ExternalInput")
    masks = nc.dram_tensor("masks", [4 * 128, TC], dt.bfloat16, kind="ExternalInput")
    outT = nc.dram_tensor("outT", [D, S], dt.float32, kind="ExternalOutput")

    with tile.TileContext(nc) as tc:
        with (
            tc.tile_pool(name="const", bufs=1) as constp,
            tc.tile_pool(name="tabs", bufs=1) as tabp,
            tc.tile_pool(name="weights", bufs=1) as wtp,
            tc.tile_pool(name="acts", bufs=1) as actp,
            tc.tile_pool(name="chunkacts", bufs=1) as cap,
            tc.tile_pool(name="xstream", bufs=2) as xsp,
            tc.tile_pool(name="scratch", bufs=3) as scr,
            tc.tile_pool(name="psum", bufs=1, space="PSUM") as psp,
        ):
            # all-ones square: the probs-sum matmul then lands the softmax
            # denominator pre-broadcast across all 128 partitions (matmul
            # cost scales with columns only, so this is free vs a [128,1])
            ones_sq = constp.tile([128, 128], dt.bfloat16, tag="ones_sq")
            nc.vector.memset(ones_sq[:], 1.0)
            # per-partition sign for the fused rope combine
            sign_t = constp.tile([128, 1], dt.float32, tag="sign")
            nc.vector.memset(sign_t[0:64, :], -1.0)
            nc.vector.memset(sign_t[64:128, :], 1.0)

            # PE warm-up: short dummy matmuls (~5us of sustained PE activity)
            # so the HAM clock gate opens to 8/8 while the first x/weight
            # DMAs are still in flight; results are never read. 1-col lhsT
            # keeps the implicit LDWEIGHTS free.
            warm_src = constp.tile([128, 64], dt.bfloat16, tag="warm")
            nc.vector.memset(warm_src[:], 0.0)
            warm1 = constp.tile([128, 1], dt.bfloat16, tag="warm1")
            nc.vector.memset(warm1[:], 1.0)
            warm_ps = psp.tile([128, TC], dt.float32, tag="mm", bufs=3)
            for _ in range(320):
                nc.tensor.matmul(warm_ps[0:1, 0:64], warm1[:], warm_src[:],
                                 start=True, stop=True, skip_group_check=True)

            # cs/masks deferred (gated on the first real matmul below) so the
            # startup HBM burst is spent on the critical wq0+x stream
            cs_t = tabp.tile([HD, S], dt.float32, tag="cs")
            cs_dma = nc.gpsimd.dma_start(cs_t[:], csT[:])
            mask_t = [tabp.tile([128, TC], dt.bfloat16, tag=f"mask{i}", name=f"mask{i}") for i in range(4)]
            for i in range(4):
                nc.gpsimd.dma_start(mask_t[i][:], masks[i * 128:(i + 1) * 128, :])

            # Resident weights. wq/wk stream on the sync ring interleaved
            # with chunk-0 x quarters; wv rides the scalar ring; wo loads
            # on gpsimd at the start of chunk-0 attention.
            wqk_t = [wtp.tile([128, C * HD], dt.bfloat16, tag=f"wqk{h}", name=f"wqk{h}")
                     for h in range(NQ + NKV)]
            wv_t = wtp.tile([128, C * NKV * HD], dt.bfloat16, tag="wv")
            wo_t = wtp.tile([128, CO * D], dt.bfloat16, tag="wo")

            wsrcs = [wqp] * NQ + [wkp] * NKV
            wcols = [h * HD for h in range(NQ)] + [h * HD for h in range(NKV)]

            def dma_wqk(h):
                src = wsrcs[h][:, wcols[h]:wcols[h] + HD]
                nc.sync.dma_start(
                    wqk_t[h].rearrange("p (c n) -> p c n", c=C),
                    src.rearrange("(c p) n -> p c n", p=128),
                )

            # K persists for the full sequence (written chunk by chunk);
            # V persists per 128-token block
            kth = [actp.tile([128, S], dt.bfloat16, tag=f"kth{h}", name=f"kth{h}") for h in range(NKV)]
            vh_t = [actp.tile([128, NKV * HD], dt.bfloat16, tag=f"vh{b}", name=f"vh{b}") for b in range(TB)]

            CQ = max(C // 4, 1)   # c-chunks per x quarter
            NG = C // CQ

            for tci in range(NTC):
                ts = slice(tci * TC, (tci + 1) * TC)
                qth = [cap.tile([128, TC], dt.bfloat16, tag=f"qth{h}", name=f"qth{h}_{tci}") for h in range(NQ)]
                oth = [cap.tile([128, TC], dt.bfloat16, tag=f"oth{h}", name=f"oth{h}_{tci}") for h in range(NQ)]

                # ---- x quarters for chunk tci ----
                # chunk 0: alternate x across sync+scalar rings for startup
                # bandwidth, interleaved with the one-time wq/wk loads; later
                # chunks prefetch on the (otherwise idle) sync ring
                wv_dma = None
                if tci == 0:
                    # chunk 0: 8 finer x pieces (256KB) alternating the
                    # sync/scalar rings so head-0 matmuls start on the first
                    # piece, interleaved with the one-time wq/wk loads
                    CQ0 = CQ // 2
                    xh_g = []
                    for g in range(2 * NG):
                        rs = slice(g * CQ0 * 128, (g + 1) * CQ0 * 128)
                        th = xsp.tile([128, CQ0 * TC], dt.bfloat16, tag="xh0",
                                      bufs=2 * NG, name=f"xh0_{g}")
                        if g == 0:
                            dma_wqk(0)
                        ring = nc.sync if g % 2 == 0 else nc.scalar
                        ring.dma_start(
                            th.rearrange("p (c n) -> p c n", c=CQ0),
                            xt[rs, ts].rearrange("(c p) n -> p c n", p=128),
                        )
                        if g == 3:
                            dma_wqk(1)
                        xh_g.append(th)
                    for h in range(2, NQ + NKV):
                        dma_wqk(h)
                    wv_dma = nc.scalar.dma_start(
                        wv_t.rearrange("p (c n) -> p c n", c=C),
                        wvp.rearrange("(c p) n -> p c n", p=128),
                    )

                    def xh_c(c, xh_g=xh_g, CQc=CQ0):
                        return xh_g[c // CQc][:, (c % CQc) * TC:(c % CQc + 1) * TC]
                else:
                    xh_g = []
                    for g in range(NG):
                        rs = slice(g * CQ * 128, (g + 1) * CQ * 128)
                        th = xsp.tile([128, CQ * TC], dt.bfloat16, tag="xh", bufs=2 * NG, name=f"xh_{tci}_{g}")
                        nc.sync.dma_start(
                            th.rearrange("p (c n) -> p c n", c=CQ),
                            xt[rs, ts].rearrange("(c p) n -> p c n", p=128),
                        )
                        xh_g.append(th)

                    def xh_c(c, xh_g=xh_g, CQc=CQ):
                        return xh_g[c // CQc][:, (c % CQc) * TC:(c % CQc + 1) * TC]

                # ---- QKV projections + RoPE ----
                for h in range(NQ + NKV):
                    is_q = h < NQ
                    ps = psp.tile([128, TC], dt.float32, tag="mm", bufs=3)
                    for c in range(C):
                        mm = nc.tensor.matmul(
                            ps[:], wqk_t[h][:, c * HD:(c + 1) * HD], xh_c(c),
                            start=(c == 0), stop=(c == C - 1),
                        )
                        if tci == 0 and h == 0 and c == 0:
                            add_dep_helper(cs_dma.ins, mm.ins,
                                           reason="defer tables past startup burst")
                            if wv_dma is not None:
                                add_dep_helper(wv_dma.ins, mm.ins,
                                               reason="defer wv past startup burst")
                    # RoPE from PSUM -> bf16 q/k tiles in 5 DVE ops: 4
                    # half-width products (PSUM input exempts the same-base
                    # SBUF rule) into A=[xr*c ; xr*s], B=[xi*s ; xi*c], then
                    # one fused full-width combine out = A + sign*B with
                    # sign = [-1]*64 ++ [+1]*64.
                    A_t = scr.tile([128, TC], dt.float32, tag="ropeA", bufs=2)
                    B_t = scr.tile([128, TC], dt.float32, tag="ropeB", bufs=2)
                    cos_s = cs_t[0:64, ts]
                    sin_s = cs_t[64:128, ts]
                    M = mybir.AluOpType
                    nc.vector.tensor_tensor(A_t[0:64, :], ps[0:64, :], cos_s, M.mult)
                    nc.vector.tensor_tensor(A_t[64:128, :], ps[0:64, :], sin_s, M.mult)
                    nc.vector.tensor_tensor(B_t[0:64, :], ps[64:128, :], sin_s, M.mult)
                    nc.vector.tensor_tensor(B_t[64:128, :], ps[64:128, :], cos_s, M.mult)
                    dst = qth[h][:] if is_q else kth[h - NQ][:, ts]
                    nc.vector.scalar_tensor_tensor(dst, B_t[:], sign_t[:], A_t[:],
                                                   M.mult, M.add)

                # V projection per 128-token block
                for tb in range(TC // 128):
                    tbg = tci * (TC // 128) + tb
                    ps = psp.tile([128, NKV * HD], dt.float32, tag="mm", bufs=3)
                    for c in range(C):
                        nc.tensor.matmul(
                            ps[:], xh_c(c)[:, tb * 128:(tb + 1) * 128],
                            wv_t[:, c * NKV * HD:(c + 1) * NKV * HD],
                            start=(c == 0), stop=(c == C - 1),
                        )
                    nc.scalar.copy(vh_t[tbg][:], ps[:])

                # ---- attention for q-chunk tci (keys 0..(tci+1)*TC) ----
                if tci == 0:
                    nc.gpsimd.dma_start(
                        wo_t.rearrange("p (c n) -> p c n", c=CO),
                        woh.rearrange("(c p) n -> p c n", p=128),
                    )
                qc = tci
                nkb = (qc + 1) * (TC // 128)
                pending_norm = []

                def emit_norm(h, ot_ps, sum_ps):
                    # sum_ps holds z broadcast across all partitions (all-ones
                    # [128,128] lhsT), so 1/z = exp(-ln z) runs on the scalar
                    # engine with all lanes busy (vector.reciprocal = ~4us/lane
                    # chain and stalls the DVE FIFO)
                    lg = scr.tile([128, TC], dt.float32, tag="lg", bufs=2, name=f"lg_{tci}_{h}")
                    nc.scalar.activation(lg[:], sum_ps[:], AF.Ln, bias=0.0, scale=1.0)
                    recb = scr.tile([128, TC], dt.float32, tag="recb", bufs=2, name=f"recb_{tci}_{h}")
                    nc.scalar.activation(recb[:], lg[:], AF.Exp, bias=0.0, scale=-1.0)
                    nc.vector.tensor_tensor(oth[h][:], ot_ps[:], recb[:], mybir.AluOpType.mult)

                # Two-stage software pipeline over all (head, block)
                # pairs: scores/exp lead PV by LAG blocks so the PE never
                # waits on the ACT/DVE probs chain at head starts.
                LAG = 6
                blocks = [(h, kb) for h in range(NQ) for kb in range(nkb)]
                head_ps = {}

                def emit_scores(h, kb):
                    kv = h // REP
                    d = kb * 128 - qc * TC
                    ks = slice(kb * 128, (kb + 1) * 128)
                    q0 = max(d, 0)
                    sc_ps = psp.tile([128, TC], dt.float32, tag="mm", bufs=3,
                                     name=f"sc_{tci}_{h}_{kb}")
                    nc.tensor.matmul(sc_ps[:, q0:TC], kth[kv][:, ks], qth[h][:, q0:TC],
                                     start=True, stop=True)
                    ph = scr.tile([128, TC], dt.bfloat16, tag="ph", bufs=LAG + 2,
                                  name=f"ph_{tci}_{h}_{kb}")
                    nc.scalar.activation(ph[:, q0:TC], sc_ps[:, q0:TC], AF.Exp, bias=0.0, scale=scale)
                    if d >= 0:
                        nc.vector.tensor_tensor(ph[:, q0:TC], ph[:, q0:TC], mask_t[d // 128][:, q0:TC], mybir.AluOpType.mult)
                    return ph

                def emit_pv(h, kb, ph):
                    kv = h // REP
                    vcol = kv * HD
                    d = kb * 128 - qc * TC
                    q0 = max(d, 0)
                    if kb == 0:
                        head_ps[h] = (
                            psp.tile([128, TC], dt.float32, tag="otps", bufs=3,
                                     name=f"ot_{tci}_{h}"),
                            psp.tile([128, TC], dt.float32, tag="sums", bufs=2,
                                     name=f"sum_{tci}_{h}"),
                        )
                    ot_ps, sum_ps = head_ps[h]
                    nc.tensor.matmul(
                        ot_ps[:, q0:TC], vh_t[kb][:, vcol:vcol + HD], ph[:, q0:TC],
                        start=(kb == 0), stop=(kb == nkb - 1),
                    )
                    nc.tensor.matmul(
                        sum_ps[:, q0:TC], ones_sq[:], ph[:, q0:TC],
                        start=(kb == 0), stop=(kb == nkb - 1),
                    )
                    if kb == nkb - 1:
                        pending_norm.append((h, ot_ps, sum_ps))
                        if len(pending_norm) > 1:
                            emit_norm(*pending_norm.pop(0))

                probs_q = []
                for h, kb in blocks:
                    probs_q.append((h, kb, emit_scores(h, kb)))
                    if len(probs_q) > LAG:
                        hh, kk, ph = probs_q.pop(0)
                        emit_pv(hh, kk, ph)
                for hh, kk, ph in probs_q:
                    emit_pv(hh, kk, ph)

                # ---- output projection for token-chunk tci ----
                # the first two dbs interleave their c=0..2 matmuls so the
                # deferred last-head norm chain (ACT Ln/Exp + DVE mult,
                # ~1.8us) is fully covered before the c=3 matmuls need oth[3]
                ps_db = {}

                def op_mm(db, c):
                    if c == 0:
                        ps_db[db] = psp.tile([128, TC], dt.float32, tag="mm", bufs=3,
                                             name=f"op_{tci}_{db}")
                    nc.tensor.matmul(
                        ps_db[db][:], wo_t[:, c * D + db * 128:c * D + (db + 1) * 128],
                        oth[c][:], start=(c == 0), stop=(c == CO - 1),
                    )

                def op_fin(db):
                    # deep buffer: the copy waits on the store-DMA completion
                    # of the tile 6 dbs back; HBM write-receipt latency (~2us)
                    # must not reach the PSUM pool
                    o3 = scr.tile([128, TC], dt.float32, tag="o3", bufs=6)
                    nc.scalar.copy(o3[:], ps_db[db][:])
                    eng = nc.sync if tci == NTC - 1 else nc.gpsimd
                    eng.dma_start(outT[db * 128:(db + 1) * 128, ts], o3[:])

                for args in pending_norm:
                    emit_norm(*args)
                pending_norm = []
                for c in range(CO - 1):
                    op_mm(0, c)
                    op_mm(1, c)
                op_mm(0, CO - 1)
                op_fin(0)
                op_mm(1, CO - 1)
                op_fin(1)
                for db in range(2, DB):
                    for c in range(CO):
                        op_mm(db, c)
                    op_fin(db)

    return nc


# ---------------------------------------------------------------------------
# walrus in this container refuses >1 sem wait per instruction ("Too many
# sync wait commands"). Hoist excess waits onto same-engine NoOps inserted
# immediately before the instruction - program order on the engine queue
# preserves the sync semantics.
def split_multiwait_insts(nc, max_waits=1):
    n_split = 0
    for bb in nc.main_func.blocks:
        insts = bb.instructions
        i = 0
        while i < len(insts):
            ins = insts[i]
            si = getattr(ins, "sync_info", None)
            if si is not None and si.on_wait and len(si.on_wait) > max_waits:
                waits = list(si.on_wait)
                head, tail = waits[:-max_waits], waits[-max_waits:]
                nops = []
                for j in range(0, len(head), max_waits):
                    nop = mybir.InstNoOp(name=f"{ins.name}-ws{j}", ins=[], outs=[])
                    nop.engine = ins.engine
                    nop.sync_info = mybir.SyncInfo(
                        on_wait=head[j:j + max_waits], on_update=[])
                    nops.append(nop)
                ins.sync_info = mybir.SyncInfo(
                    on_wait=tail, on_update=list(si.on_update or []))
                insts[i:i] = nops
                i += len(nops)
                n_split += 1
            i += 1
    return n_split


# ---------------------------------------------------------------------------
# Host-side shard preparation / gather
BF16 = ml_dtypes.bfloat16


def rope_tables(S, HD):
    inv = 1.0 / (10000.0 ** (np.arange(0, HD, 2, dtype=np.float32) / HD))
    t = np.arange(S, dtype=np.float32)
    f = np.outer(t, inv).astype(np.float32)  # [S, HD//2]
    return np.ascontiguousarray(np.cos(f).T), np.ascontiguousarray(np.sin(f).T)


def causal_masks(TC):
    # masks[dd][k, qrel] = 1 if k + dd*128 <= qrel else 0
    out = np.zeros((4 * 128, TC), BF16)
    k = np.arange(128)[:, None]
    q = np.arange(TC)[None, :]
    for dd in range(4):
        out[dd * 128:(dd + 1) * 128] = (k + dd * 128 <= q).astype(BF16)
    return out


def rope_perm(HD):
    # new row i (i < HD//2) = old 2i; new row HD//2+i = old 2i+1
    return np.concatenate([np.arange(0, HD, 2), np.arange(1, HD, 2)])


def make_in_maps(x, wq, wk, wv, wo, *, n_batch_shards, n_head_shards,
                 NQ_TOT, NKV_TOT, HD, TC):
    """Returns list of in_maps, one per core (batch-major: core = b*G + g)."""
    B, S, D = x.shape
    G = n_head_shards
    NQ = NQ_TOT // G
    NKV = NKV_TOT // G
    perm = rope_perm(HD)
    cosT, sinT = rope_tables(S, HD)
    csT = np.concatenate([cosT, sinT], axis=0)   # [HD, S]
    masks = causal_masks(TC)

    # Per-batch xT in bf16 (shared across head shards)
    xt = {}
    for b in range(B):
        xt[b] = np.ascontiguousarray(x[b].T).astype(BF16)  # [D, S]

    # Per-headgroup weight shards
    wshard = {}
    for g in range(G):
        qrows = slice(g * NQ * HD, (g + 1) * NQ * HD)
        kvrows = slice(g * NKV * HD, (g + 1) * NKV * HD)
        wq_g = wq[qrows, :].copy()      # [NQ*HD, D]
        wk_g = wk[kvrows, :].copy()
        wv_g = wv[kvrows, :].copy()
        # RoPE permutation of output rows, per head
        for hh in range(NQ):
            blk = wq_g[hh * HD:(hh + 1) * HD]
            wq_g[hh * HD:(hh + 1) * HD] = blk[perm]
        for hh in range(NKV):
            blk = wk_g[hh * HD:(hh + 1) * HD]
            wk_g[hh * HD:(hh + 1) * HD] = blk[perm]
        wqT = np.ascontiguousarray(wq_g.T).astype(BF16)   # [D, NQ*HD]
        wkT = np.ascontiguousarray(wk_g.T).astype(BF16)
        wvT = np.ascontiguousarray(wv_g.T).astype(BF16)
        woT = np.ascontiguousarray(wo[:, qrows].T).astype(BF16)  # [NQ*HD, D]
        wshard[g] = (wqT, wkT, wvT, woT)

    in_maps = []
    for b in range(n_batch_shards):
        for g in range(G):
            wqT, wkT, wvT, woT = wshard[g]
            in_maps.append({
                "xt": xt[b],
                "wqp": wqT, "wkp": wkT, "wvp": wvT,
                "woh": woT,
                "csT": csT,
                "masks": masks,
            })
    return in_maps


def combine_outputs(outTs, B, G):
    """outTs: list of [D, S] partials, core order b*G+g. Returns [B, S, D]."""
    outs = []
    for b in range(B):
        acc = outTs[b * G].astype(np.float32).copy()
        for g in range(1, G):
            acc += outTs[b * G + g]
        outs.append(acc.T)  # [S, D]
    return np.stack(outs)


_NC_CACHE = {}


def _get_nc(S, D, NQ, NKV, HD, TC):
    key = (S, D, NQ, NKV, HD, TC)
    if key not in _NC_CACHE:
        nc = build_attention_nc(S=S, D=D, NQ=NQ, NKV=NKV, HD=HD, TC=TC)
        split_multiwait_insts(nc)
        _NC_CACHE[key] = nc
    return _NC_CACHE[key]


def kernel(**inputs):
    x = np.asarray(inputs["x"], dtype=np.float32)
    wq = np.asarray(inputs["wq"], dtype=np.float32)
    wk = np.asarray(inputs["wk"], dtype=np.float32)
    wv = np.asarray(inputs["wv"], dtype=np.float32)
    wo = np.asarray(inputs["wo"], dtype=np.float32)

    B, S, D = x.shape          # (2, 2048, 2048)
    NQ_TOT = wq.shape[0] // 128
    NKV_TOT = wk.shape[0] // 128
    HD = 128
    TC = 512
    G = 4                      # head shards
    NQ, NKV = NQ_TOT // G, NKV_TOT // G

    nc = _get_nc(S, D, NQ, NKV, HD, TC)
    in_maps = make_in_maps(
        x, wq, wk, wv, wo,
        n_batch_shards=B, n_head_shards=G,
        NQ_TOT=NQ_TOT, NKV_TOT=NKV_TOT, HD=HD, TC=TC,
    )

    from concourse.bass_utils import run_bass_kernel_spmd

    trace = os.environ.get("BASS_ATTN_TRACE") == "1"
    res = run_bass_kernel_spmd(nc, in_maps, list(range(len(in_maps))), trace=trace)
    kernel.last_results = res
    outTs = [r["outT"] for r in res.results]
    return combine_outputs(outTs, B, G).astype(np.float32)
